# revision 13
# baseline (speedup 1.0000x reference)
"""Local self-attention (window=65) Trainium2 kernel, 8 NeuronCores.

Sharding: 4096 tokens (B*S flattened) split into 8 shards of 512 tokens.
Each core gets a halo'd, pre-transposed x slice plus replicated weights
(halo = 32 tokens each side, zero-padded at batch-sequence edges; zero x
tokens produce exactly-zero k/v since the qkv projection has no bias,
matching the reference's zero-padding semantics).

Per-core pipeline (Bass/Tile, bf16 matmuls with fp32 accumulation),
transposed-score orientation — no weight transposes:
  1. qT/kT projections (feature-major) and v projection (token-major,
     packed [tok, 8 heads x (64 v | 1.0)] so a ones-column rides along
     as AV's denominator).
  2. Per (head, 256-query bpair): banded scores computed TRANSPOSED
     [keys, queries] in a tight [128, 512] layout (3 key-chunk matmuls:
     A keys[q0,q0+128) x q 0:128, BC keys[q0+128,q0+256) x q 0:256,
     D keys[q0+256,q0+384) x q 128:256), one ACT exp, then a 0/1 band
     mask MULTIPLY on DVE (bf16 2x) — no PE mask matmuls. AV: 4 matmuls
     with em chunks stationary -> token-major attn [q, 64+den].
     Per-partition reciprocal + tensor_scalar normalize.
  3. Per 128-token block: 4 PE transposes of the normalized attn back
     to feature-major, output projection + bias, DMA out (split across
     two queues).
  PE is pre-warmed with dummy matmuls during the DMA load phase; input
  DMAs are issued from four different engine queues in parallel so the
  weight stream starts as early as possible.
"""

import numpy as np
import ml_dtypes

import concourse.bass as bass
import concourse.mybir as mybir
import concourse.tile as tile
from concourse import bacc
from concourse.bass_utils import run_bass_kernel_spmd

F32 = mybir.dt.float32
BF16 = mybir.dt.bfloat16

# ---- problem constants (hardcoded) ----
B, S, DM = 2, 2048, 512
H, D, WIN = 8, 64, 65
PAD = WIN // 2              # 32
NCORES = 8
SHARD = B * S // NCORES     # 512 tokens per core
HALO = SHARD + 2 * PAD      # 576
NFT = DM // 128             # 4 feature tiles
NQB = 2                     # query block-pairs of 256
QBS = 256                   # queries per bpair
EMW = 512                   # em cols: A 0:128 | BC 128:384 | D 384:512


def _np_dt(dt):
    return {F32: np.float32, BF16: ml_dtypes.bfloat16}[dt]


def _build_program():
    nc = bacc.Bacc("TRN2", target_bir_lowering=False, debug=False)

    # tile-packed layouts: [128, nft*cols] with feature-tile kc at
    # cols [kc*cols, (kc+1)*cols)
    xT_d = nc.dram_tensor("xT", [128, NFT * HALO], BF16, kind="ExternalInput")
    Wq_d = nc.dram_tensor("Wq", [128, NFT * DM], BF16, kind="ExternalInput")
    Wk_d = nc.dram_tensor("Wk", [128, NFT * DM], BF16, kind="ExternalInput")
    Wv_d = nc.dram_tensor("Wv", [128, NFT * DM], BF16, kind="ExternalInput")
    Wo_d = nc.dram_tensor("Wo", [128, NFT * DM], BF16, kind="ExternalInput")
    bias_d = nc.dram_tensor("bias", [DM], F32, kind="ExternalInput")
    mi_d = nc.dram_tensor("maskident", [128, EMW + 128], BF16,
                          kind="ExternalInput")
    out_d = nc.dram_tensor("out", [SHARD, DM], BF16,
                           kind="ExternalOutput")

    Exp = mybir.ActivationFunctionType.Exp
    Copy = mybir.ActivationFunctionType.Copy

    with tile.TileContext(nc) as tc:
        with (
            tc.tile_pool(name="consts", bufs=1) as cpool,
            tc.tile_pool(name="qkv", bufs=1) as qpool,
            tc.tile_pool(name="work", bufs=4) as wpool,
            tc.tile_pool(name="outp", bufs=2) as opool,
            tc.tile_pool(name="pp", bufs=2, space="PSUM") as pp,
            tc.tile_pool(name="sc", bufs=2, space="PSUM") as scp,
            tc.tile_pool(name="av", bufs=2, space="PSUM") as avp,
        ):
            # ---- loads: two HWDGE queues (sync + scalar); per-queue
            # issue order matches stream order = first-use order.
            xT_all = cpool.tile([128, NFT * HALO], BF16, tag="xT")
            Wq_all = cpool.tile([128, NFT * DM], BF16, tag="Wq")
            Wk_all = cpool.tile([128, NFT * DM], BF16, tag="Wk")
            Wv_all = cpool.tile([128, NFT * DM], BF16, tag="Wv")
            Wo_all = cpool.tile([128, NFT * DM], BF16, tag="Wo")
            mi_sb = cpool.tile([128, EMW + 128], BF16, tag="mi")
            bias_sb = cpool.tile([128, DM], F32, tag="bias")
            hx = NFT * HALO // 2
            hw = NFT * DM // 2

            def load2(dram, tile_, cols):
                h = cols // 2
                nc.sync.dma_start(out=tile_[:, 0:h], in_=dram[:, 0:h])
                nc.scalar.dma_start(out=tile_[:, h:cols], in_=dram[:, h:cols])

            hq = NFT * DM // 2
            load2(xT_d, xT_all, NFT * HALO)
            load2(Wq_d, Wq_all, NFT * DM)
            load2(Wk_d, Wk_all, NFT * DM)
            nc.scalar.dma_start(out=mi_sb[:], in_=mi_d[:, :])
            nc.sync.dma_start(out=Wv_all[:, 0:hq], in_=Wv_d[:, 0:hq])
            nc.scalar.dma_start(out=Wv_all[:, hq:], in_=Wv_d[:, hq:])
            bias_ap = bias_d[:]
            nc.scalar.dma_start(
                out=bias_sb[:],
                in_=bass.AP(tensor=bias_ap.tensor, offset=bias_ap.offset,
                            ap=[[0, 128]] + list(bias_ap.ap)),
            )
            load2(Wo_d, Wo_all, NFT * DM)

            # ---- PE warm-up: dummy matmuls during the DMA wait keep
            # the HAM activity window busy so real matmuls run at 2.4GHz
            warm = cpool.tile([128, 512], BF16, tag="warm")
            nc.gpsimd.memset(warm[:], 0.0)
            wps = pp.tile([128, 512], F32, tag="pp", name="warmps")
            for i in range(16):
                nc.tensor.matmul(wps[:, 0:384], warm[:, 0:128],
                                 warm[:, 0:384], start=True, stop=True)

            xT_sb = [xT_all[:, kc * HALO:(kc + 1) * HALO] for kc in range(NFT)]
            Wq_sb = [Wq_all[:, kc * DM:(kc + 1) * DM] for kc in range(NFT)]
            Wk_sb = [Wk_all[:, kc * DM:(kc + 1) * DM] for kc in range(NFT)]
            Wv_sb = [Wv_all[:, kc * DM:(kc + 1) * DM] for kc in range(NFT)]
            Wo_sb = [Wo_all[:, kc * DM:(kc + 1) * DM] for kc in range(NFT)]
            mask_sb = mi_sb[:, 0:EMW]
            ident_sb = mi_sb[:, EMW:EMW + 128]

            # ---- projections ----
            qT_sb = [None] * NFT
            kT_sb = [None] * NFT
            v_sb = [None] * 5      # [tok, 8*(64 v | 1.0)] per 128-tok tile

            def emit_qk(ft):
                csl = slice(ft * 128, ft * 128 + 128)
                psq = pp.tile([128, 512], F32, tag="pp", name=f"psq{ft}")
                for kc in range(NFT):
                    nc.tensor.matmul(
                        psq[:], Wq_sb[kc][:, csl], xT_sb[kc][:, PAD:PAD + SHARD],
                        start=(kc == 0), stop=(kc == NFT - 1))
                qt = qpool.tile([128, SHARD], BF16, tag=f"qT{ft}",
                                name=f"qT{ft}")
                nc.vector.tensor_copy(out=qt[:], in_=psq[:])
                qT_sb[ft] = qt

                kt = qpool.tile([128, 640], BF16, tag=f"kT{ft}",
                                name=f"kT{ft}")
                nc.gpsimd.memset(kt[:, HALO:640], 0.0)
                psk = pp.tile([128, 512], F32, tag="pp", name=f"psk{ft}")
                for kc in range(NFT):
                    nc.tensor.matmul(
                        psk[:], Wk_sb[kc][:, csl], xT_sb[kc][:, 0:512],
                        start=(kc == 0), stop=(kc == NFT - 1))
                nc.scalar.activation(out=kt[:, 0:512], in_=psk[:], func=Copy)
                psk2 = avp.tile([128, 64], F32, tag="av", name=f"psk2{ft}")
                for kc in range(NFT):
                    nc.tensor.matmul(
                        psk2[:], Wk_sb[kc][:, csl], xT_sb[kc][:, 512:HALO],
                        start=(kc == 0), stop=(kc == NFT - 1))
                nc.scalar.activation(out=kt[:, 512:HALO], in_=psk2[:],
                                     func=Copy)
                kT_sb[ft] = kt

            def emit_v(tt):
                rows = 128 if tt < 4 else HALO - 512
                psv = pp.tile([128, 512], F32, tag="pp", name=f"psv{tt}")
                for kc in range(NFT):
                    nc.tensor.matmul(
                        psv[:rows, :], xT_sb[kc][:, tt * 128:tt * 128 + rows],
                        Wv_sb[kc][:, :],
                        start=(kc == 0), stop=(kc == NFT - 1))
                vt = qpool.tile([128, H * (D + 1)], BF16, tag=f"v{tt}",
                                name=f"v{tt}")
                if rows < 128:
                    nc.gpsimd.memset(vt[rows:128, :], 0.0)
                vv = vt[:].rearrange("p (h c) -> p h c", h=H)
                nc.gpsimd.memset(vv[:, :, D:D + 1], 1.0)
                nc.scalar.activation(
                    out=vv[:rows, :, 0:D],
                    in_=psv[:rows, :].rearrange("p (h c) -> p h c", h=H),
                    func=Copy)
                v_sb[tt] = vt

            attn_tok = [qpool.tile([128, DM], BF16, tag=f"at{b}",
                                   name=f"at{b}") for b in range(4)]
            attnT_sb = [qpool.tile([128, SHARD], BF16, tag=f"attnT{i}",
                                   name=f"attnT{i}") for i in range(NFT)]

            # ---- attention stages: head-PAIR fronts (pair j covers
            # iterations i=2j, 2j+1 = heads 2hp, 2hp+1 of one qb).
            # The two heads' K=64 stationaries sit on partition halves
            # 0:64 / 64:128, so adjacent emission runs them on different
            # PE row-groups concurrently (ldw of one overlaps mm of the
            # other). One [128,1024] sc pair-tile -> single merged exp.
            def front2(j):
                qb, hp = divmod(j, H // 2)
                kt, qt = kT_sb[hp], qT_sb[hp]
                q0 = qb * QBS
                sc = scp.tile([128, 1024], F32, tag="sc", name=f"sc{j}")
                if j >= 6:
                    # warm-keeper: dense filler so HAM stays at 8/8
                    # through the tail; start=True re-clears after.
                    nc.tensor.matmul(sc[:, 0:512], warm[:, 0:128],
                                     warm[:, 0:512], start=True, stop=True)
                    nc.tensor.matmul(sc[:, 512:1024], warm[:, 0:128],
                                     warm[:, 0:512], start=True, stop=True)
                # A: keys[q0,q0+128) x q 0:128 ; BC: keys[q0+128,q0+256)
                # x q 0:256 ; D: keys[q0+256,q0+384) x q 128:256.
                for hh, co in ((0, 0), (1, 512)):
                    rsl = slice(hh * 64, hh * 64 + 64)
                    nc.tensor.matmul(sc[:, co:co + 128],
                                     kt[rsl, q0:q0 + 128],
                                     qt[rsl, q0:q0 + 128],
                                     start=True, stop=False)
                for hh, co in ((0, 0), (1, 512)):
                    rsl = slice(hh * 64, hh * 64 + 64)
                    nc.tensor.matmul(sc[:, co + 128:co + 384],
                                     kt[rsl, q0 + 128:q0 + 256],
                                     qt[rsl, q0:q0 + 256],
                                     start=False, stop=False)
                for hh, co in ((0, 0), (1, 512)):
                    rsl = slice(hh * 64, hh * 64 + 64)
                    nc.tensor.matmul(sc[:, co + 384:co + 512],
                                     kt[rsl, q0 + 256:q0 + 384],
                                     qt[rsl, q0 + 128:q0 + 256],
                                     start=False, stop=False)
                # additive band mask (identity matmul accumulates -1e4
                # onto out-of-band entries); exp underflows them to 0
                nc.tensor.matmul(sc[:, 0:512], ident_sb[:],
                                 mask_sb[:, 0:512], start=False, stop=True)
                nc.tensor.matmul(sc[:, 512:1024], ident_sb[:],
                                 mask_sb[:, 0:512], start=False, stop=True)
                em = wpool.tile([128, 2 * EMW], BF16, tag="em", name=f"em{j}")
                nc.scalar.activation(out=em[:], in_=sc[:], func=Exp,
                                     scale=0.125)
                return em

            def back(i, em, eo):
                qb, h = divmod(i, H)
                hs = slice(h * (D + 1), (h + 1) * (D + 1))
                av = avp.tile([128, 512], F32, tag="av", name=f"av{i}")
                nc.tensor.matmul(av[:, 0:D + 1],
                                 em[0:128, eo:eo + 128],
                                 v_sb[2 * qb][0:128, hs],
                                 start=True, stop=False)
                nc.tensor.matmul(av[:, 0:D + 1],
                                 em[0:128, eo + 128:eo + 256],
                                 v_sb[2 * qb + 1][0:128, hs],
                                 start=False, stop=True)
                nc.tensor.matmul(av[:, 128:128 + D + 1],
                                 em[0:128, eo + 256:eo + 384],
                                 v_sb[2 * qb + 1][0:128, hs],
                                 start=True, stop=False)
                nc.tensor.matmul(av[:, 128:128 + D + 1],
                                 em[0:128, eo + 384:eo + 512],
                                 v_sb[2 * qb + 2][0:128, hs],
                                 start=False, stop=True)
                # normalize per qhalf: per-partition recip of den column
                for qh in range(2):
                    blk = 2 * qb + qh
                    co = qh * 128
                    rcp = wpool.tile([128, 1], F32, tag="rcp",
                                     name=f"rcp{i}_{qh}")
                    nc.vector.reciprocal_approx_fast(
                        out=rcp[:], in_=av[:, co + D:co + D + 1])
                    nc.vector.tensor_scalar_mul(
                        attn_tok[blk][:, h * D:(h + 1) * D],
                        av[:, co:co + D], rcp[:, 0:1])

            def emit_T(blk, ft):
                tp = pp.tile([128, 128], BF16, tag="pp", name=f"tp{blk}_{ft}")
                nc.tensor.transpose(
                    tp[:], attn_tok[blk][:, ft * 128:(ft + 1) * 128],
                    ident_sb[:])
                if ft % 2 == 0:
                    nc.vector.tensor_copy(
                        out=attnT_sb[ft][:, blk * 128:(blk + 1) * 128],
                        in_=tp[:])
                else:
                    nc.scalar.activation(
                        out=attnT_sb[ft][:, blk * 128:(blk + 1) * 128],
                        in_=tp[:], func=Copy)

            def po_mms(blk, po, kcs, start, stop):
                for kc in kcs:
                    nc.tensor.matmul(
                        po[:], attnT_sb[kc][:, blk * 128:(blk + 1) * 128],
                        Wo_sb[kc][:, :],
                        start=(start and kc == kcs[0]),
                        stop=(stop and kc == kcs[-1]))

            def po_fin(blk, po):
                osb = opool.tile([128, DM], BF16, tag="osb", name=f"osb{blk}")
                nc.vector.tensor_add(osb[:], po[:], bias_sb[:])
                oeng = nc.sync if blk % 2 == 0 else nc.scalar
                oeng2 = nc.scalar if blk % 2 == 0 else nc.sync
                oeng.dma_start(out=out_d[blk * 128:(blk + 1) * 128, 0:256],
                               in_=osb[:, 0:256])
                oeng2.dma_start(out=out_d[blk * 128:(blk + 1) * 128, 256:512],
                                in_=osb[:, 256:512])

            def emit_po(blk):
                po = pp.tile([128, 512], F32, tag="pp", name=f"po{blk}")
                po_mms(blk, po, [0, 1, 2, 3], True, True)
                po_fin(blk, po)

            # ---- main schedule: software-pipelined with skew 1 ----
            def bridge(n, name):
                # dense PE filler into a throwaway sc tile: keeps the HAM
                # activity window busy across load-wait stalls
                sd = scp.tile([128, 1024], F32, tag="sc", name=name)
                for j in range(n):
                    nc.tensor.matmul(sd[:, 0:512], warm[:, 0:128],
                                     warm[:, 0:512], start=True, stop=True)

            emit_qk(0)
            emit_qk(1)

            def block_out(blk):
                for ft in range(NFT):
                    emit_T(blk, ft)
                emit_po(blk)

            extras = {0: lambda: emit_qk(2), 2: lambda: emit_qk(3),
                      4: lambda: emit_v(3), 5: lambda: emit_v(4),
                      8: lambda: block_out(0), 9: lambda: block_out(1)}
            NIT = NQB * H
            fr = [front2(0), front2(1)]
            bridge(2, "br0")
            for tt in range(3):
                emit_v(tt)
            po2 = po3 = None
            for i in range(NIT):
                if i in extras:
                    extras[i]()
                if i % 2 == 0 and i // 2 + 2 < NIT // 2:
                    fr.append(front2(i // 2 + 2))
                if i in (10, 12, 14):
                    ft = (i - 10) // 2
                    emit_T(2, ft)
                    emit_T(3, ft)
                if i == 15:
                    # out-proj for block 2, kc 0..2 (attnT 0..2 ready)
                    po2 = pp.tile([128, 512], F32, tag="pp", name="po2")
                    po_mms(2, po2, [0, 1, 2], True, False)
                back(i, fr[i // 2], (i % 2) * EMW)
            bridge(1, "br1")
            emit_T(2, 3)
            po_mms(2, po2, [3], False, True)
            po_fin(2, po2)
            emit_T(3, 3)
            po3 = pp.tile([128, 512], F32, tag="pp", name="po3")
            po_mms(3, po3, [0, 1, 2, 3], True, True)
            po_fin(3, po3)

    nc.compile()
    return nc


_CACHE = {}


def _get_program():
    if "nc" not in _CACHE:
        _CACHE["nc"] = _build_program()
    return _CACHE["nc"]


def _make_in_maps(x, W_qkv, W_out, b_out):
    npb = _np_dt(BF16)

    def pack(M, dt):
        # [512, C] -> [128, 4*C] with row-tile kc at cols [kc*C, (kc+1)*C)
        return np.ascontiguousarray(
            np.concatenate([M[128 * kc:128 * (kc + 1)] for kc in range(NFT)],
                           axis=1), dtype=dt)

    Wr = W_qkv.reshape(DM, H, 3, D)
    Wq = pack(Wr[:, :, 0, :].reshape(DM, DM), npb)
    Wk = pack(Wr[:, :, 1, :].reshape(DM, DM), npb)
    Wv = pack(Wr[:, :, 2, :].reshape(DM, DM), npb)
    Wo = pack(W_out, npb)
    bias = np.ascontiguousarray(b_out, dtype=np.float32)

    # transposed 0/1 band mask [keys, queries] for the tight em layout:
    #   A  @ [0:128]   keys kk = halo q0+kk,     q = q0+qq   (qq 0:128)
    #   BC @ [128:384] keys kk = halo q0+128+kk, q = q0+qq   (qq 0:256)
    #   D  @ [384:512] keys kk = halo q0+256+kk, q = q0+128+qq (qq 0:128)
    # in-band iff |key_token - q| <= 32 with key_token = halo_idx - 32.
    kk = np.arange(128)[:, None]
    qqA = np.arange(128)[None, :]
    mA = (kk - qqA >= 0) & (kk - qqA <= 64)
    qqBC = np.arange(256)[None, :]
    mBC = (kk >= qqBC - 128) & (kk <= qqBC - 64)
    mD = mBC[:, 0:128]
    band = np.concatenate([mA, mBC, mD], axis=1)
    maskT = np.where(band, 0.0, -10000.0)
    maskident = np.ascontiguousarray(
        np.concatenate([maskT, np.eye(128)], axis=1), dtype=npb)

    in_maps = []
    for c in range(NCORES):
        bidx, s0 = c // (NCORES // B), (c % (NCORES // B)) * SHARD
        xh = np.zeros((HALO, DM), np.float32)
        lo, hi = s0 - PAD, s0 + SHARD + PAD
        clo, chi = max(lo, 0), min(hi, S)
        xh[clo - lo:chi - lo] = x[bidx, clo:chi]
        xT = pack(np.ascontiguousarray(xh.T), npb)
        in_maps.append({
            "xT": xT, "Wq": Wq, "Wk": Wk, "Wv": Wv, "Wo": Wo,
            "bias": bias, "maskident": maskident,
        })
    return in_maps


def kernel(x, W_qkv, W_out, b_out, _trace=False, _tmpdir=None):
    x = np.asarray(x, dtype=np.float32)
    W_qkv = np.asarray(W_qkv, dtype=np.float32)
    W_out = np.asarray(W_out, dtype=np.float32)
    b_out = np.asarray(b_out, dtype=np.float32)

    nc = _get_program()
    in_maps = _make_in_maps(x, W_qkv, W_out, b_out)
    res = run_bass_kernel_spmd(
        nc, in_maps, list(range(NCORES)), trace=_trace, tmpdir=_tmpdir)
    _CACHE["last_results"] = res
    out = np.concatenate(
        [res.results[c]["out"] for c in range(NCORES)], axis=0)
    return out.reshape(B, S, DM).astype(np.float32)
